# revision 36
# baseline (speedup 1.0000x reference)
"""Viterbi decode (CRF) kernel for Trainium2, data-parallel over batch on 8 cores.

Forward recurrence per core (128 batch rows on partitions):
    carry_i[b, c] = max_p(carry_{i-1}[b, p] + trans[p, c]) + feat_i[b, c]
computed as a DVE broadcast-add into a (c-major, p-inner) score tile followed by a
segmented reduce_max, preserving the reference's fp32 rounding order exactly.
All carries are kept in SBUF; the backtrace recomputes the argmax only along the
traced path (trans column select via 4 concurrent 32x32 PE matmuls on a one-hot,
then DVE max/max_index).
"""

import sys

sys.path.insert(0, "/opt/trn_rl_repo")

import numpy as np

from concourse import bass, mybir
from concourse.tile import TileContext

F32 = mybir.dt.float32
I32 = mybir.dt.int32
U32 = mybir.dt.uint32

B_LOC = 128  # batch rows per core
T = 32  # tags
N_CORES = 8


def build_nc(S: int, chunk: int = 128, fix_waits: bool = True):
    nc = bass.Bass()

    feats_d = nc.declare_dram_parameter("feats", [B_LOC, S, T], F32, isOutput=False)
    consts_d = nc.declare_dram_parameter("consts", [B_LOC, 36 * T], F32, isOutput=False)
    path_d = nc.declare_dram_parameter("path", [B_LOC, S], I32, isOutput=True)

    add = mybir.AluOpType.add
    maxop = mybir.AluOpType.max
    iseq = mybir.AluOpType.is_equal

    with TileContext(nc) as tc:
        with (
            tc.tile_pool(name="const", bufs=1) as cpool,
            tc.tile_pool(name="featp", bufs=1) as fpool,
            tc.tile_pool(name="work", bufs=1) as wpool,
            tc.tile_pool(name="psum", bufs=2, space="PSUM") as ppool,
        ):
            consts_t = cpool.tile([B_LOC, 36 * T], F32)
            nc.sync.dma_start(out=consts_t[:], in_=consts_d[:])
            transT = consts_t[:, 0 : T * T].rearrange("p (c q) -> p c q", q=T)
            transmm = consts_t[:, T * T : T * T + T]
            start_t = consts_t[:, T * T + T : T * T + 2 * T]
            stop_t = consts_t[:, T * T + 2 * T : T * T + 3 * T]
            iota_t = consts_t[:, T * T + 3 * T : T * T + 4 * T]

            carries = wpool.tile([B_LOC, S, T], F32)
            path_t = wpool.tile([B_LOC, S], I32)
            scores = wpool.tile([B_LOC, T, T], F32)
            red = wpool.tile([B_LOC, T], F32)
            final = wpool.tile([B_LOC, T], F32)
            max8 = wpool.tile([B_LOC, 8], F32)
            idx8 = wpool.tile([B_LOC, 8], U32)
            cur = wpool.tile([B_LOC, 1], F32)
            oh = wpool.tile([B_LOC, T], F32)
            ohT = wpool.tile([B_LOC, T], F32)
            Fsb = wpool.tile([B_LOC, T], F32)

            # The compute-instruction encodings fit a single sync-wait, but an
            # op reading two DMA'd tensors would need waits on two HW-DMA
            # queues. Touch each DMA'd tile once on DVE (one wait per touch,
            # each with its own destination tile so no WAW chain); later DVE
            # consumers then inherit the ordering via engine program order.
            # The dummy matmul does the same for the PE engine.
            ft = fpool.tile([B_LOC, S, T], F32)
            nc.sync.dma_start(out=ft[:], in_=feats_d[:])
            tt0 = wpool.tile([B_LOC, 1], F32, tag="touch0")
            nc.vector.tensor_copy(tt0[:], consts_t[:, 0:1])
            tt1 = wpool.tile([B_LOC, 1], F32, tag="touch1")
            nc.vector.tensor_copy(tt1[:], ft[:, 0, 0:1])
            tt2 = wpool.tile([B_LOC, 1], F32, tag="touch2")
            nc.gpsimd.tensor_copy(tt2[:], consts_t[:, 0:1])
            Fp0 = ppool.tile([B_LOC, T], F32, tag="Fpdummy")
            nc.tensor.matmul(
                Fp0[0:32, :],
                transmm[0:32, :],
                transmm[0:32, :],
                start=True,
                stop=True,
                tile_position=(0, 0),
            )

            # ---------------- forward ----------------
            # Per-step work is split three ways: GPSIMD (a second,
            # concurrent vector-class engine) computes the broadcast-add for
            # the last G tags into its own scores tile while DVE adds the
            # first 24 in two interleaved slices (so each op's pipeline drain
            # hides behind its independent sibling). DVE then reduces its own
            # slices first — by the time it reaches the GPSIMD slice, that
            # add has landed.
            G = 16  # tags delegated to GPSIMD
            D = T - G  # tags on DVE
            Dh = D // 2
            scores_g = wpool.tile([B_LOC, G, T], F32)
            nc.vector.tensor_tensor(carries[:, 0, :], ft[:, 0, :], start_t, op=add)
            for i in range(1, S):
                cprev_g = (
                    carries[:, i - 1, :].unsqueeze(1).broadcast_to([B_LOC, G, T])
                )
                nc.gpsimd.tensor_tensor(
                    scores_g[:], cprev_g, transT[:, D:T, :], op=add
                )
                cprev = carries[:, i - 1, :].unsqueeze(1).broadcast_to([B_LOC, Dh, T])
                nc.vector.tensor_tensor(
                    scores[:, 0:Dh, :], cprev, transT[:, 0:Dh, :], op=add
                )
                nc.vector.tensor_tensor(
                    scores[:, Dh:D, :], cprev, transT[:, Dh:D, :], op=add
                )
                nc.vector.tensor_reduce(
                    red[:, 0:Dh], scores[:, 0:Dh, :], axis=mybir.AxisListType.X, op=maxop
                )
                nc.vector.tensor_reduce(
                    red[:, Dh:D], scores[:, Dh:D, :], axis=mybir.AxisListType.X, op=maxop
                )
                nc.vector.tensor_reduce(
                    red[:, D:T], scores_g[:], axis=mybir.AxisListType.X, op=maxop
                )
                nc.vector.tensor_tensor(
                    carries[:, i, 0:D], red[:, 0:D], ft[:, i, 0:D], op=add
                )
                nc.vector.tensor_tensor(
                    carries[:, i, D:T], red[:, D:T], ft[:, i, D:T], op=add
                )

            # ---------------- final tag ----------------
            nc.vector.tensor_tensor(final[:], carries[:, S - 1, :], stop_t, op=add)
            nc.vector.max(max8[:], final[:])
            nc.vector.max_index(idx8[:], max8[:], final[:])
            nc.vector.tensor_copy(path_t[:, S - 1 : S], idx8[:, 0:1])

            # ---------------- backtrace ----------------
            for i in range(S - 1, 0, -1):
                nc.vector.tensor_tensor(
                    oh[:],
                    iota_t,
                    idx8[:, 0:1].broadcast_to([B_LOC, T]),
                    op=iseq,
                )
                nc.vector.transpose(ohT[:], oh[:])
                Fp = ppool.tile([B_LOC, T], F32, tag="Fp")
                for k in range(4):
                    nc.tensor.matmul(
                        Fp[32 * k : 32 * k + 32, :],
                        ohT[32 * k : 32 * k + 32, :],
                        transmm[32 * k : 32 * k + 32, :],
                        start=True,
                        stop=True,
                        tile_position=(32 * k, 32 * k),
                    )
                nc.vector.tensor_tensor(
                    Fsb[:], carries[:, i - 1, :], Fp[:], op=add
                )
                nc.vector.max(max8[:], Fsb[:])
                nc.vector.max_index(idx8[:], max8[:], Fsb[:])
                nc.vector.tensor_copy(path_t[:, i - 1 : i], idx8[:, 0:1])

            nc.sync.dma_start(out=path_d[:], in_=path_t[:])

    if fix_waits:
        _strip_redundant_pe_waits(nc)
    return nc


def _strip_redundant_pe_waits(nc):
    """Walrus encodes at most one sync-wait per compute instruction. The
    backtrace matmuls carry [PE >= a (PSUM WAW), DVE >= b (one-hot ready)];
    the PE wait is transitively implied whenever some DVE instruction with
    completion tick <= b already waited on PE >= a (Tile doesn't do
    transitive minimization across procs). Verify that implication from the
    vector clocks, then drop the PE wait."""
    f = nc.m.functions[0]
    insts = [i for blk in f.blocks for i in blk.instructions]

    # Cumulative: after the k-th DVE-sem increment, the largest value of each
    # other engine's semaphore that the DVE engine has waited on so far.
    # "DVE >= k completed" then implies "sem X >= that value".
    dve_tick = 0
    observed = {}  # sem prefix -> list of (dve_tick, max value waited by DVE)
    cur_max = {}
    for inst in insts:
        si = inst.sync_info
        if si is None:
            continue
        if str(inst.engine).endswith("DVE"):
            for w in si.on_wait or []:
                if w.ant_name and not w.ant_name.startswith("DVE_"):
                    p = w.ant_name.split("_")[0]
                    cur_max[p] = max(cur_max.get(p, 0), w.wait_value)
        for u in si.on_update or []:
            if u.ant_name and u.ant_name.startswith("DVE_"):
                dve_tick += u.update_value
                for p, v in cur_max.items():
                    observed.setdefault(p, []).append((dve_tick, v))

    def pe_implied(dve_val, other_name, other_val):
        p = other_name.split("_")[0]
        best = 0
        for k, v in observed.get(p, []):
            if k <= dve_val:
                best = max(best, v)
        return best >= other_val

    from concourse import mybir as _mybir
    import copy as _copy

    # Drains (CTRL_NO struct) also fit a single wait: split any multi-wait
    # drain into a chain of single-wait drains on the same engine.
    for blk in f.blocks:
        new_list = []
        for inst in blk.instructions:
            si = inst.sync_info
            if (
                type(inst).__name__ == "InstDrain"
                and si is not None
                and si.on_wait
                and len(si.on_wait) > 1
            ):
                waits = list(si.on_wait)
                for k, w in enumerate(waits[:-1]):
                    clone = _copy.copy(inst)
                    clone.name = f"{inst.name}-w{k}"
                    clone.sync_info = _mybir.SyncInfo(on_wait=[w], on_update=[])
                    new_list.append(clone)
                inst.sync_info = _mybir.SyncInfo(
                    on_wait=[waits[-1]], on_update=list(si.on_update or [])
                )
            new_list.append(inst)
        blk.instructions[:] = new_list

    n_stripped = 0
    for inst in insts:
        si = inst.sync_info
        if si is None or not si.on_wait or len(si.on_wait) <= 1:
            continue
        pe_waits = [w for w in si.on_wait if not w.ant_name.startswith("DVE_")]
        dve_waits = [w for w in si.on_wait if w.ant_name.startswith("DVE_")]
        if len(si.on_wait) == 2 and len(pe_waits) == 1 and len(dve_waits) == 1:
            if pe_implied(
                dve_waits[0].wait_value, pe_waits[0].ant_name, pe_waits[0].wait_value
            ):
                inst.sync_info = _mybir.SyncInfo(
                    on_wait=dve_waits, on_update=list(si.on_update or [])
                )
                n_stripped += 1
    remaining = [
        (i.name, type(i).__name__, [(w.ant_name, w.wait_value) for w in i.sync_info.on_wait])
        for i in insts
        if i.sync_info
        and i.sync_info.on_wait
        and len(i.sync_info.on_wait) > 1
        and type(i).__name__
        not in ("InstDrain", "InstEventSemaphore", "InstISA", "InstCall")
    ]
    if remaining:
        raise RuntimeError(f"unresolvable multi-wait instructions: {remaining[:5]}")


def _make_const_inputs(transitions, start_transitions, stop_transitions):
    transitions = np.asarray(transitions, dtype=np.float32)
    start = np.asarray(start_transitions, dtype=np.float32)
    stop = np.asarray(stop_transitions, dtype=np.float32)
    consts = np.zeros((B_LOC, 36 * T), dtype=np.float32)
    consts[:, : T * T] = transitions.T.reshape(1, T * T)  # [c*32+p] = trans[p,c]
    consts[:, T * T : T * T + T] = np.tile(transitions.T, (4, 1))  # transmm
    consts[:, T * T + T : T * T + 2 * T] = start[None, :]
    consts[:, T * T + 2 * T : T * T + 3 * T] = stop[None, :]
    consts[:, T * T + 3 * T : T * T + 4 * T] = np.arange(T, dtype=np.float32)[None, :]
    return {"consts": consts}


class Runner:
    """Compile once, keep inputs device-resident, execute repeatedly."""

    def __init__(self, nc, n_cores=N_CORES):
        import jax
        from jax.sharding import Mesh, PartitionSpec, NamedSharding
        from jax.experimental.shard_map import shard_map
        from concourse import bass2jax

        self.jax = jax
        bass2jax.install_neuronx_cc_hook()

        partition_name = (
            nc.partition_id_tensor.name if nc.partition_id_tensor else None
        )
        in_names, out_names, out_avals, zero_outs = [], [], [], []
        for alloc in nc.m.functions[0].allocations:
            if not isinstance(alloc, mybir.MemoryLocationSet):
                continue
            name = alloc.memorylocations[0].name
            if alloc.kind == "ExternalInput":
                if name != partition_name:
                    in_names.append(name)
            elif alloc.kind == "ExternalOutput":
                out_names.append(name)
                shape = tuple(alloc.tensor_shape)
                dtype = mybir.dt.np(alloc.dtype)
                out_avals.append(jax.core.ShapedArray(shape, dtype))
                zero_outs.append(np.zeros((n_cores * shape[0], *shape[1:]), dtype))
        n_params = len(in_names)
        all_names = in_names + out_names
        if partition_name is not None:
            all_names = all_names + [partition_name]

        def _body(*args):
            operands = list(args)
            if partition_name is not None:
                operands.append(bass2jax.partition_id_tensor())
            outs = bass2jax._bass_exec_p.bind(
                *operands,
                out_avals=tuple(out_avals),
                in_names=tuple(all_names),
                out_names=tuple(out_names),
                lowering_input_output_aliases=(),
                sim_require_finite=True,
                sim_require_nnan=True,
                nc=nc,
            )
            return tuple(outs)

        self._body = _body
        devices = jax.devices()[:n_cores]
        assert len(devices) == n_cores
        self.mesh = Mesh(np.asarray(devices), ("core",))
        in_specs = (PartitionSpec("core"),) * (n_params + len(out_names))
        out_specs = (PartitionSpec("core"),) * len(out_names)
        self.sharded = jax.jit(
            shard_map(
                _body,
                mesh=self.mesh,
                in_specs=in_specs,
                out_specs=out_specs,
                check_rep=False,
            ),
            donate_argnums=tuple(range(n_params, n_params + len(out_names))),
            keep_unused=True,
        )
        self.sharding = NamedSharding(self.mesh, PartitionSpec("core"))
        self.in_names = in_names
        self.out_names = out_names
        self.out_avals = out_avals
        self.zero_outs = zero_outs
        self.n_cores = n_cores
        self.dev_in = None

    def set_inputs(self, in_maps):
        concat = [
            np.concatenate([np.asarray(m[name]) for m in in_maps], axis=0)
            for name in self.in_names
        ]
        self.dev_in = [self.jax.device_put(a, self.sharding) for a in concat]

    def execute(self):
        outs = self.sharded(*self.dev_in, *[z.copy() for z in self.zero_outs])
        outs = self.jax.block_until_ready(outs)
        return {
            name: np.asarray(outs[i]).reshape(
                self.n_cores, *self.out_avals[i].shape
            )
            for i, name in enumerate(self.out_names)
        }

    def make_chained(self, n_chain):
        """Callable dispatching the NEFF n_chain times, each execution's
        outputs threaded in as the next one's output-seed operands (data
        dependency serializes them on device); blocks once at the end.
        Wall-time slope over n_chain isolates on-device execution time from
        per-call host/RPC overhead."""
        import jax
        from jax.experimental.shard_map import shard_map
        from jax.sharding import PartitionSpec

        n_params = len(self.in_names)
        in_specs = (PartitionSpec("core"),) * (n_params + len(self.out_names))
        out_specs = (PartitionSpec("core"),) * len(self.out_names)
        fn = jax.jit(
            shard_map(
                self._body,
                mesh=self.mesh,
                in_specs=in_specs,
                out_specs=out_specs,
                check_rep=False,
            ),
            keep_unused=True,
        )
        dev_zeros = [self.jax.device_put(z, self.sharding) for z in self.zero_outs]

        def run():
            outs = tuple(dev_zeros)
            for _ in range(n_chain):
                outs = fn(*self.dev_in, *outs)
            return self.jax.block_until_ready(outs)

        return run


_RUNNER_CACHE = {}


def _get_runner(S, kind="main"):
    key = (S, kind)
    if key not in _RUNNER_CACHE:
        nc = build_nc(S) if kind == "main" else build_noop_nc(S)
        _RUNNER_CACHE[key] = Runner(nc)
    return _RUNNER_CACHE[key]


def build_noop_nc(S):
    """Same I/O signature, near-zero device work — for launch-overhead calibration."""
    nc = bass.Bass()
    nc.declare_dram_parameter("feats", [B_LOC, S, T], F32, isOutput=False)
    consts_d = nc.declare_dram_parameter("consts", [B_LOC, 36 * T], F32, isOutput=False)
    path_d = nc.declare_dram_parameter("path", [B_LOC, S], I32, isOutput=True)
    with TileContext(nc) as tc:
        with tc.tile_pool(name="w", bufs=1) as pool:
            t = pool.tile([B_LOC, T], F32)
            nc.sync.dma_start(out=t[:], in_=consts_d[:, 0:T])
            ti = pool.tile([B_LOC, T], I32)
            nc.vector.tensor_copy(ti[:], t[:])
            nc.sync.dma_start(out=path_d[:, 0:T], in_=ti[:])
    _strip_redundant_pe_waits(nc)
    return nc


def _in_maps_for(feats, transitions, start_transitions, stop_transitions, n_cores):
    consts = _make_const_inputs(transitions, start_transitions, stop_transitions)
    in_maps = []
    for c in range(n_cores):
        m = dict(consts)
        m["feats"] = np.ascontiguousarray(feats[c * B_LOC : (c + 1) * B_LOC])
        in_maps.append(m)
    return in_maps


def run_on_cores(feats, transitions, start_transitions, stop_transitions, trace=False):
    feats = np.asarray(feats, dtype=np.float32)
    B, S, T_ = feats.shape
    assert T_ == T and B % B_LOC == 0
    n_cores = B // B_LOC
    runner = _get_runner(S)
    runner.set_inputs(
        _in_maps_for(feats, transitions, start_transitions, stop_transitions, n_cores)
    )
    out = runner.execute()["path"]
    return out.reshape(B, S).astype(np.int32), None


def kernel(feats, tags, transitions, start_transitions, stop_transitions):
    out, _ = run_on_cores(feats, transitions, start_transitions, stop_transitions)
    return out
